# revision 4
# baseline (speedup 1.0000x reference)
"""GraphSAGE 2-layer kernel for 8 Trainium2 NeuronCores.

Strategy (graph/data parallel, dst-partitioned):
  - Relabel nodes: degree-sorted serpentine deal into 392 blocks of 128 nodes
    so every block has ~equal total in-degree -> uniform edge-tile count T per
    block -> one SPMD program for all 8 cores (49 blocks/core).
  - Pre-project features before the gather (segment_sum commutes with the
    linear map): p = h @ W_neigh computed per-core on its own node shard,
    AllGather'd to a full [N_pad, 64] table -> gathers move 64-wide rows.
  - Per 128-edge tile: indirect-DMA gather of p[src] rows, build a one-hot
    [edge, dst-slot] selection matrix on DVE (is_equal vs iota), and
    scatter-add via PE matmul accumulating into PSUM per 128-dst block.
  - h = relu(x @ W_self + inv_deg * agg + b), second layer identical with an
    on-chip PE transpose to feed h1^T as lhsT.
"""

import numpy as np

N = 50000
E = 800000
IN_F, HID_F, OUT_F = 128, 64, 64
CORES = 8
P = 128
NB = 392          # total dst blocks
BPC = NB // CORES  # 49 blocks per core
R = BPC * P        # 6272 rows per core
NPAD = NB * P      # 50176

_cache = {}


def _prep(x, src, dst):
    """Host-side sharding: relabel nodes, build per-core padded edge tiles."""
    deg = np.bincount(dst, minlength=N).astype(np.int64)
    inv_deg = (1.0 / np.maximum(deg, 1)).astype(np.float32)

    # serpentine deal of degree-sorted nodes into NB blocks -> balanced edges
    order = np.argsort(-deg, kind="stable").astype(np.int64)
    idx = np.arange(N, dtype=np.int64)
    rnd = idx // NB
    k = idx % NB
    b_of = np.where(rnd % 2 == 0, k, NB - 1 - k)
    blk = np.empty(N, np.int64)
    slot = np.empty(N, np.int64)
    blk[order] = b_of
    slot[order] = rnd
    pos = blk * P + slot                      # old id -> new id
    old_of_new = np.full(NPAD, -1, np.int64)
    old_of_new[pos] = idx

    nsrc = pos[src.astype(np.int64)]
    ndst = pos[dst.astype(np.int64)]
    B = ndst >> 7
    dslot = ndst & 127

    o = np.argsort(B, kind="stable")
    Bs = B[o]
    s_s = nsrc[o].astype(np.int32)
    d_s = dslot[o].astype(np.float32)
    counts = np.bincount(Bs, minlength=NB)
    T = int(np.ceil(counts.max() / P))
    cap = T * P
    starts = np.zeros(NB + 1, np.int64)
    np.cumsum(counts, out=starts[1:])
    rank = np.arange(E, dtype=np.int64) - starts[Bs]

    src_pad = np.zeros((NB, cap), np.int32)
    dst_pad = np.full((NB, cap), 200.0, np.float32)
    src_pad[Bs, rank] = s_s
    dst_pad[Bs, rank] = d_s

    # per-core tensors
    xp = np.zeros((NPAD, IN_F), np.float32)
    xp[: N if False else NPAD] = 0.0
    valid = old_of_new >= 0
    xp[valid] = x[old_of_new[valid]]
    invd_new = np.ones(NPAD, np.float32)
    invd_new[valid] = inv_deg[old_of_new[valid]]

    percore = []
    for c in range(CORES):
        bs, be = c * BPC, (c + 1) * BPC
        srcT = src_pad[bs:be].reshape(BPC * T, P).T.copy()          # [128, 49T]
        dstT = dst_pad[bs:be].reshape(BPC * T, P).T.copy()          # [128, 49T]
        xT = xp[c * R : (c + 1) * R].T.copy()                       # [128, R]
        invd = invd_new[c * R : (c + 1) * R].reshape(BPC, P).T.copy()  # [128, 49]
        percore.append((srcT, dstT, xT, invd))
    return T, pos, percore


def _build(T, has_b1, has_b2):
    """Build + compile the SPMD bass program (uniform across cores)."""
    import concourse.bacc as bacc
    import concourse.bass as bass
    import concourse.mybir as mybir
    import concourse.tile as tile

    f32 = mybir.dt.float32
    i32 = mybir.dt.int32
    NT = BPC * T

    nc = bacc.Bacc("TRN2", target_bir_lowering=False, debug=False,
                   num_devices=CORES)

    xT_d = nc.dram_tensor("xT", [P, R], f32, kind="ExternalInput")
    srcT_d = nc.dram_tensor("srcT", [P, NT], i32, kind="ExternalInput")
    dstT_d = nc.dram_tensor("dstT", [P, NT], f32, kind="ExternalInput")
    invd_d = nc.dram_tensor("invd", [P, BPC], f32, kind="ExternalInput")
    w1n_d = nc.dram_tensor("w1n", [IN_F, HID_F], f32, kind="ExternalInput")
    w1s_d = nc.dram_tensor("w1s", [IN_F, HID_F], f32, kind="ExternalInput")
    w2n_d = nc.dram_tensor("w2n", [HID_F, OUT_F], f32, kind="ExternalInput")
    w2s_d = nc.dram_tensor("w2s", [HID_F, OUT_F], f32, kind="ExternalInput")
    iota_d = nc.dram_tensor("iota", [P, P], f32, kind="ExternalInput")
    b1_d = nc.dram_tensor("b1r", [P, HID_F], f32, kind="ExternalInput")
    b2_d = nc.dram_tensor("b2r", [P, OUT_F], f32, kind="ExternalInput")
    out_d = nc.dram_tensor("out", [R, OUT_F], f32, kind="ExternalOutput")

    cc_in1 = nc.dram_tensor("cc_in1", [R, HID_F], f32)
    cc_out1 = nc.dram_tensor("cc_out1", [NPAD, HID_F], f32)
    cc_in2 = nc.dram_tensor("cc_in2", [R, HID_F], f32)
    cc_out2 = nc.dram_tensor("cc_out2", [NPAD, HID_F], f32)

    groups = [list(range(CORES))]
    eq = mybir.AluOpType.is_equal
    mul = mybir.AluOpType.mult
    relu = mybir.ActivationFunctionType.Relu

    from concourse.masks import make_identity

    with tile.TileContext(nc) as tc:
        with (
            tc.tile_pool(name="pers", bufs=1) as pers,
            tc.tile_pool(name="gath", bufs=8) as gpool,
            tc.tile_pool(name="sel", bufs=6) as spool,
            tc.tile_pool(name="stage", bufs=4) as stage,
            tc.tile_pool(name="pagg", bufs=2, space="PSUM") as pagg_pool,
            tc.tile_pool(name="pself", bufs=2, space="PSUM") as pself_pool,
            tc.tile_pool(name="ptr", bufs=2, space="PSUM") as ptr_pool,
        ):
            xT = pers.tile([P, R], f32)
            nc.sync.dma_start(out=xT[:], in_=xT_d[:, :])
            srcT = pers.tile([P, NT], i32)
            nc.sync.dma_start(out=srcT[:], in_=srcT_d[:, :])
            dstT = pers.tile([P, NT], f32)
            nc.sync.dma_start(out=dstT[:], in_=dstT_d[:, :])
            invd = pers.tile([P, BPC], f32)
            nc.sync.dma_start(out=invd[:], in_=invd_d[:, :])
            w1n = pers.tile([IN_F, HID_F], f32)
            nc.sync.dma_start(out=w1n[:], in_=w1n_d[:, :])
            w1s = pers.tile([IN_F, HID_F], f32)
            nc.sync.dma_start(out=w1s[:], in_=w1s_d[:, :])
            w2n = pers.tile([HID_F, OUT_F], f32)
            nc.sync.dma_start(out=w2n[:], in_=w2n_d[:, :])
            w2s = pers.tile([HID_F, OUT_F], f32)
            nc.sync.dma_start(out=w2s[:], in_=w2s_d[:, :])
            iota = pers.tile([P, P], f32)
            nc.sync.dma_start(out=iota[:], in_=iota_d[:, :])
            b1r = pers.tile([P, HID_F], f32)
            if has_b1:
                nc.sync.dma_start(out=b1r[:], in_=b1_d[:, :])
            b2r = pers.tile([P, OUT_F], f32)
            if has_b2:
                nc.sync.dma_start(out=b2r[:], in_=b2_d[:, :])
            ident = pers.tile([P, P], f32)
            make_identity(nc, ident[:])
            h1 = pers.tile([P, BPC * HID_F], f32)
            h1T = pers.tile([HID_F, R], f32)

            # ---- phase B: p1 shard = x @ W1_neigh, block by block -> cc_in1
            for b in range(BPC):
                ps = pagg_pool.tile([P, HID_F], f32, tag="proj")
                nc.tensor.matmul(out=ps[:], lhsT=xT[:, b * P : (b + 1) * P],
                                 rhs=w1n[:], start=True, stop=True)
                t = stage.tile([P, HID_F], f32, tag="proj_sb")
                nc.vector.tensor_copy(out=t[:], in_=ps[:])
                nc.sync.dma_start(out=cc_in1[b * P : (b + 1) * P, :], in_=t[:])

            nc.gpsimd.collective_compute(
                "AllGather", mybir.AluOpType.bypass, replica_groups=groups,
                ins=[cc_in1.ap().opt()], outs=[cc_out1.ap().opt()])

            # ---- layers
            def layer(cc_out, w_self, self_lhsT, has_b, br, emit):
                for b in range(BPC):
                    pg = pagg_pool.tile([P, HID_F], f32, tag="agg")
                    for j in range(T):
                        ti = b * T + j
                        g = gpool.tile([P, HID_F], f32, tag="g")
                        nc.gpsimd.indirect_dma_start(
                            out=g[:], out_offset=None, in_=cc_out[:, :],
                            in_offset=bass.IndirectOffsetOnAxis(
                                ap=srcT[:, ti : ti + 1], axis=0))
                        s = spool.tile([P, P], f32, tag="s")
                        nc.vector.tensor_tensor(
                            out=s[:], in0=dstT[:, ti : ti + 1].to_broadcast([P, P]),
                            in1=iota[:], op=eq)
                        nc.tensor.matmul(out=pg[:], lhsT=s[:], rhs=g[:],
                                         start=(j == 0), stop=(j == T - 1))
                    pf = pself_pool.tile([P, HID_F], f32, tag="self")
                    nc.tensor.matmul(out=pf[:], lhsT=self_lhsT(b), rhs=w_self[:],
                                     start=True, stop=True)
                    tmp = stage.tile([P, HID_F], f32, tag="c1")
                    nc.vector.tensor_tensor(
                        out=tmp[:], in0=pg[:],
                        in1=invd[:, b : b + 1].to_broadcast([P, HID_F]), op=mul)
                    tmp2 = stage.tile([P, HID_F], f32, tag="c2")
                    nc.vector.tensor_add(out=tmp2[:], in0=tmp[:], in1=pf[:])
                    if has_b:
                        tmp3 = stage.tile([P, HID_F], f32, tag="c3")
                        nc.vector.tensor_add(out=tmp3[:], in0=tmp2[:], in1=br[:])
                        tmp2 = tmp3
                    emit(b, tmp2)

            # layer 1: emit h1 block + transposed copy, then p2 proj -> cc_in2
            def emit1(b, tmp2):
                nc.scalar.activation(out=h1[:, b * HID_F : (b + 1) * HID_F],
                                     in_=tmp2[:], func=relu)
                pt = ptr_pool.tile([HID_F, P], f32, tag="tr")
                nc.tensor.transpose(out=pt[:],
                                    in_=h1[:, b * HID_F : (b + 1) * HID_F],
                                    identity=ident[:])
                nc.vector.tensor_copy(out=h1T[:, b * P : (b + 1) * P], in_=pt[:])
                ps = pagg_pool.tile([P, HID_F], f32, tag="proj")
                nc.tensor.matmul(out=ps[:], lhsT=h1T[:, b * P : (b + 1) * P],
                                 rhs=w2n[:], start=True, stop=True)
                t = stage.tile([P, HID_F], f32, tag="proj_sb")
                nc.vector.tensor_copy(out=t[:], in_=ps[:])
                nc.sync.dma_start(out=cc_in2[b * P : (b + 1) * P, :], in_=t[:])

            layer(cc_out1, w1s, lambda b: xT[:, b * P : (b + 1) * P],
                  has_b1, b1r, emit1)

            nc.gpsimd.collective_compute(
                "AllGather", mybir.AluOpType.bypass, replica_groups=groups,
                ins=[cc_in2.ap().opt()], outs=[cc_out2.ap().opt()])

            def emit2(b, tmp2):
                ob = stage.tile([P, OUT_F], f32, tag="ob")
                nc.scalar.activation(out=ob[:], in_=tmp2[:], func=relu)
                nc.sync.dma_start(out=out_d[b * P : (b + 1) * P, :], in_=ob[:])

            layer(cc_out2, w2s, lambda b: h1T[:, b * P : (b + 1) * P],
                  has_b2, b2r, emit2)

    nc.compile()
    return nc


def _run(inputs, trace=False, tmpdir=None):
    from concourse.bass_utils import run_bass_kernel_spmd

    x = np.asarray(inputs["x"], np.float32)
    src = np.asarray(inputs["src"])
    dst = np.asarray(inputs["dst"])
    T, pos, percore = _prep(x, src, dst)
    b1 = np.asarray(inputs["b1"], np.float32)
    b2 = np.asarray(inputs["b2"], np.float32)
    has_b1 = bool(np.any(b1))
    has_b2 = bool(np.any(b2))

    key = (T, has_b1, has_b2)
    if key not in _cache:
        _cache[key] = _build(T, has_b1, has_b2)
    nc = _cache[key]

    iota = np.broadcast_to(np.arange(P, dtype=np.float32), (P, P)).copy()
    shared = {
        "w1n": np.asarray(inputs["W1_neigh"], np.float32),
        "w1s": np.asarray(inputs["W1_self"], np.float32),
        "w2n": np.asarray(inputs["W2_neigh"], np.float32),
        "w2s": np.asarray(inputs["W2_self"], np.float32),
        "iota": iota,
        "b1r": np.broadcast_to(b1, (P, HID_F)).copy(),
        "b2r": np.broadcast_to(b2, (P, OUT_F)).copy(),
    }
    in_maps = []
    for c in range(CORES):
        srcT, dstT, xT, invd = percore[c]
        m = dict(shared)
        m.update({"srcT": srcT, "dstT": dstT, "xT": xT, "invd": invd})
        in_maps.append(m)

    res = run_bass_kernel_spmd(nc, in_maps, list(range(CORES)),
                               trace=trace, tmpdir=tmpdir)
    h2_new = np.concatenate([res.results[c]["out"] for c in range(CORES)], axis=0)
    out = h2_new[pos]
    return out.astype(np.float32), res


def kernel(**inputs) -> np.ndarray:
    out, _ = _run(inputs, trace=False)
    return out


# revision 5
# speedup vs baseline: 1.0097x; 1.0097x over previous
"""GraphSAGE 2-layer kernel for 8 Trainium2 NeuronCores.

Strategy (graph/data parallel, dst-partitioned):
  - Relabel nodes: degree-sorted serpentine deal into 392 blocks of 128 nodes
    so every block has ~equal total in-degree -> uniform edge-tile count T per
    block -> one SPMD program for all 8 cores (49 blocks/core).
  - Pre-project features before the gather (segment_sum commutes with the
    linear map): p = h @ W_neigh computed per-core on its own node shard,
    AllGather'd to a full [N_pad, 64] table -> gathers move 64-wide rows.
  - Per 128-edge tile: indirect-DMA gather of p[src] rows, build a one-hot
    [edge, dst-slot] selection matrix on DVE (is_equal vs iota), and
    scatter-add via PE matmul accumulating into PSUM per 128-dst block.
  - h = relu(x @ W_self + inv_deg * agg + b), second layer identical with an
    on-chip PE transpose to feed h1^T as lhsT.
"""

import numpy as np

N = 50000
E = 800000
IN_F, HID_F, OUT_F = 128, 64, 64
CORES = 8
P = 128
NB = 392          # total dst blocks
BPC = NB // CORES  # 49 blocks per core
R = BPC * P        # 6272 rows per core
NPAD = NB * P      # 50176

_cache = {}


def _prep(x, src, dst):
    """Host-side sharding: relabel nodes, build per-core padded edge tiles."""
    deg = np.bincount(dst, minlength=N).astype(np.int64)
    inv_deg = (1.0 / np.maximum(deg, 1)).astype(np.float32)

    # serpentine deal of degree-sorted nodes into NB blocks -> balanced edges
    order = np.argsort(-deg, kind="stable").astype(np.int64)
    idx = np.arange(N, dtype=np.int64)
    rnd = idx // NB
    k = idx % NB
    b_of = np.where(rnd % 2 == 0, k, NB - 1 - k)
    blk = np.empty(N, np.int64)
    slot = np.empty(N, np.int64)
    blk[order] = b_of
    slot[order] = rnd
    pos = blk * P + slot                      # old id -> new id
    old_of_new = np.full(NPAD, -1, np.int64)
    old_of_new[pos] = idx

    nsrc = pos[src.astype(np.int64)]
    ndst = pos[dst.astype(np.int64)]
    B = ndst >> 7
    dslot = ndst & 127

    o = np.argsort(B, kind="stable")
    Bs = B[o]
    s_s = nsrc[o].astype(np.int32)
    d_s = dslot[o].astype(np.float32)
    counts = np.bincount(Bs, minlength=NB)
    T = int(np.ceil(counts.max() / P))
    cap = T * P
    starts = np.zeros(NB + 1, np.int64)
    np.cumsum(counts, out=starts[1:])
    rank = np.arange(E, dtype=np.int64) - starts[Bs]

    src_pad = np.zeros((NB, cap), np.int32)
    dst_pad = np.full((NB, cap), 200.0, np.float32)
    src_pad[Bs, rank] = s_s
    dst_pad[Bs, rank] = d_s

    # per-core tensors
    xp = np.zeros((NPAD, IN_F), np.float32)
    xp[: N if False else NPAD] = 0.0
    valid = old_of_new >= 0
    xp[valid] = x[old_of_new[valid]]
    invd_new = np.ones(NPAD, np.float32)
    invd_new[valid] = inv_deg[old_of_new[valid]]

    percore = []
    for c in range(CORES):
        bs, be = c * BPC, (c + 1) * BPC
        srcT = src_pad[bs:be].reshape(BPC * T, P).T.copy()          # [128, 49T]
        dstT = dst_pad[bs:be].reshape(BPC * T, P).T.copy()          # [128, 49T]
        xT = xp[c * R : (c + 1) * R].T.copy()                       # [128, R]
        invd = invd_new[c * R : (c + 1) * R].reshape(BPC, P).T.copy()  # [128, 49]
        percore.append((srcT, dstT, xT, invd))
    return T, pos, percore


def _build(T, has_b1, has_b2):
    """Build + compile the SPMD bass program (uniform across cores)."""
    import concourse.bacc as bacc
    import concourse.bass as bass
    import concourse.mybir as mybir
    import concourse.tile as tile

    f32 = mybir.dt.float32
    bf16 = mybir.dt.bfloat16
    i32 = mybir.dt.int32
    NT = BPC * T

    nc = bacc.Bacc("TRN2", target_bir_lowering=False, debug=False,
                   num_devices=CORES)

    xT_d = nc.dram_tensor("xT", [P, R], f32, kind="ExternalInput")
    srcT_d = nc.dram_tensor("srcT", [P, NT], i32, kind="ExternalInput")
    dstT_d = nc.dram_tensor("dstT", [P, NT], f32, kind="ExternalInput")
    invd_d = nc.dram_tensor("invd", [P, BPC], f32, kind="ExternalInput")
    w1n_d = nc.dram_tensor("w1n", [IN_F, HID_F], f32, kind="ExternalInput")
    w1s_d = nc.dram_tensor("w1s", [IN_F, HID_F], f32, kind="ExternalInput")
    w2n_d = nc.dram_tensor("w2n", [HID_F, OUT_F], f32, kind="ExternalInput")
    w2s_d = nc.dram_tensor("w2s", [HID_F, OUT_F], f32, kind="ExternalInput")
    iota_d = nc.dram_tensor("iota", [P, P], f32, kind="ExternalInput")
    b1_d = nc.dram_tensor("b1r", [P, HID_F], f32, kind="ExternalInput")
    b2_d = nc.dram_tensor("b2r", [P, OUT_F], f32, kind="ExternalInput")
    out_d = nc.dram_tensor("out", [R, OUT_F], f32, kind="ExternalOutput")

    cc_in1 = nc.dram_tensor("cc_in1", [R, HID_F], bf16)
    cc_out1 = nc.dram_tensor("cc_out1", [NPAD, HID_F], bf16)
    cc_in2 = nc.dram_tensor("cc_in2", [R, HID_F], bf16)
    cc_out2 = nc.dram_tensor("cc_out2", [NPAD, HID_F], bf16)

    groups = [list(range(CORES))]
    eq = mybir.AluOpType.is_equal
    mul = mybir.AluOpType.mult
    relu = mybir.ActivationFunctionType.Relu

    from concourse.masks import make_identity

    with tile.TileContext(nc) as tc:
        with (
            tc.tile_pool(name="pers", bufs=1) as pers,
            tc.tile_pool(name="gath", bufs=8) as gpool,
            tc.tile_pool(name="sel", bufs=6) as spool,
            tc.tile_pool(name="stage", bufs=4) as stage,
            tc.tile_pool(name="pagg", bufs=2, space="PSUM") as pagg_pool,
            tc.tile_pool(name="pself", bufs=2, space="PSUM") as pself_pool,
            tc.tile_pool(name="ptr", bufs=2, space="PSUM") as ptr_pool,
        ):
            xT = pers.tile([P, R], f32)
            nc.sync.dma_start(out=xT[:], in_=xT_d[:, :])
            srcT = pers.tile([P, NT], i32)
            nc.sync.dma_start(out=srcT[:], in_=srcT_d[:, :])
            dstT = pers.tile([P, NT], f32)
            nc.sync.dma_start(out=dstT[:], in_=dstT_d[:, :])
            invd = pers.tile([P, BPC], f32)
            nc.sync.dma_start(out=invd[:], in_=invd_d[:, :])
            w1n = pers.tile([IN_F, HID_F], f32)
            nc.sync.dma_start(out=w1n[:], in_=w1n_d[:, :])
            w1s = pers.tile([IN_F, HID_F], f32)
            nc.sync.dma_start(out=w1s[:], in_=w1s_d[:, :])
            w2n = pers.tile([HID_F, OUT_F], f32)
            nc.sync.dma_start(out=w2n[:], in_=w2n_d[:, :])
            w2s = pers.tile([HID_F, OUT_F], f32)
            nc.sync.dma_start(out=w2s[:], in_=w2s_d[:, :])
            iota = pers.tile([P, P], f32)
            nc.sync.dma_start(out=iota[:], in_=iota_d[:, :])
            b1r = pers.tile([P, HID_F], f32)
            if has_b1:
                nc.sync.dma_start(out=b1r[:], in_=b1_d[:, :])
            b2r = pers.tile([P, OUT_F], f32)
            if has_b2:
                nc.sync.dma_start(out=b2r[:], in_=b2_d[:, :])
            ident = pers.tile([P, P], f32)
            make_identity(nc, ident[:])
            h1 = pers.tile([P, BPC * HID_F], f32)
            h1T = pers.tile([HID_F, R], f32)

            # ---- phase B: p1 shard = x @ W1_neigh, block by block -> cc_in1
            for b in range(BPC):
                ps = pagg_pool.tile([P, HID_F], f32, tag="proj")
                nc.tensor.matmul(out=ps[:], lhsT=xT[:, b * P : (b + 1) * P],
                                 rhs=w1n[:], start=True, stop=True)
                t = stage.tile([P, HID_F], bf16, tag="proj_sb")
                nc.vector.tensor_copy(out=t[:], in_=ps[:])
                nc.sync.dma_start(out=cc_in1[b * P : (b + 1) * P, :], in_=t[:])

            nc.gpsimd.collective_compute(
                "AllGather", mybir.AluOpType.bypass, replica_groups=groups,
                ins=[cc_in1.ap().opt()], outs=[cc_out1.ap().opt()])

            # ---- layers
            def layer(cc_out, w_self, self_lhsT, has_b, br, emit):
                for b in range(BPC):
                    pg = pagg_pool.tile([P, HID_F], f32, tag="agg")
                    for j in range(T):
                        ti = b * T + j
                        g = gpool.tile([P, HID_F], bf16, tag="g")
                        nc.gpsimd.indirect_dma_start(
                            out=g[:], out_offset=None, in_=cc_out[:, :],
                            in_offset=bass.IndirectOffsetOnAxis(
                                ap=srcT[:, ti : ti + 1], axis=0))
                        s = spool.tile([P, P], bf16, tag="s")
                        nc.vector.tensor_tensor(
                            out=s[:], in0=dstT[:, ti : ti + 1].to_broadcast([P, P]),
                            in1=iota[:], op=eq)
                        nc.tensor.matmul(out=pg[:], lhsT=s[:], rhs=g[:],
                                         start=(j == 0), stop=(j == T - 1))
                    pf = pself_pool.tile([P, HID_F], f32, tag="self")
                    nc.tensor.matmul(out=pf[:], lhsT=self_lhsT(b), rhs=w_self[:],
                                     start=True, stop=True)
                    tmp = stage.tile([P, HID_F], f32, tag="c1")
                    nc.vector.tensor_tensor(
                        out=tmp[:], in0=pg[:],
                        in1=invd[:, b : b + 1].to_broadcast([P, HID_F]), op=mul)
                    tmp2 = stage.tile([P, HID_F], f32, tag="c2")
                    nc.vector.tensor_add(out=tmp2[:], in0=tmp[:], in1=pf[:])
                    if has_b:
                        tmp3 = stage.tile([P, HID_F], f32, tag="c3")
                        nc.vector.tensor_add(out=tmp3[:], in0=tmp2[:], in1=br[:])
                        tmp2 = tmp3
                    emit(b, tmp2)

            # layer 1: emit h1 block + transposed copy, then p2 proj -> cc_in2
            def emit1(b, tmp2):
                nc.scalar.activation(out=h1[:, b * HID_F : (b + 1) * HID_F],
                                     in_=tmp2[:], func=relu)
                pt = ptr_pool.tile([HID_F, P], f32, tag="tr")
                nc.tensor.transpose(out=pt[:],
                                    in_=h1[:, b * HID_F : (b + 1) * HID_F],
                                    identity=ident[:])
                nc.vector.tensor_copy(out=h1T[:, b * P : (b + 1) * P], in_=pt[:])
                ps = pagg_pool.tile([P, HID_F], f32, tag="proj")
                nc.tensor.matmul(out=ps[:], lhsT=h1T[:, b * P : (b + 1) * P],
                                 rhs=w2n[:], start=True, stop=True)
                t = stage.tile([P, HID_F], bf16, tag="proj_sb")
                nc.vector.tensor_copy(out=t[:], in_=ps[:])
                nc.sync.dma_start(out=cc_in2[b * P : (b + 1) * P, :], in_=t[:])

            layer(cc_out1, w1s, lambda b: xT[:, b * P : (b + 1) * P],
                  has_b1, b1r, emit1)

            nc.gpsimd.collective_compute(
                "AllGather", mybir.AluOpType.bypass, replica_groups=groups,
                ins=[cc_in2.ap().opt()], outs=[cc_out2.ap().opt()])

            def emit2(b, tmp2):
                ob = stage.tile([P, OUT_F], f32, tag="ob")
                nc.scalar.activation(out=ob[:], in_=tmp2[:], func=relu)
                nc.sync.dma_start(out=out_d[b * P : (b + 1) * P, :], in_=ob[:])

            layer(cc_out2, w2s, lambda b: h1T[:, b * P : (b + 1) * P],
                  has_b2, b2r, emit2)

    nc.compile()
    return nc


def _run(inputs, trace=False, tmpdir=None):
    from concourse.bass_utils import run_bass_kernel_spmd

    x = np.asarray(inputs["x"], np.float32)
    src = np.asarray(inputs["src"])
    dst = np.asarray(inputs["dst"])
    T, pos, percore = _prep(x, src, dst)
    b1 = np.asarray(inputs["b1"], np.float32)
    b2 = np.asarray(inputs["b2"], np.float32)
    has_b1 = bool(np.any(b1))
    has_b2 = bool(np.any(b2))

    key = (T, has_b1, has_b2)
    if key not in _cache:
        _cache[key] = _build(T, has_b1, has_b2)
    nc = _cache[key]

    iota = np.broadcast_to(np.arange(P, dtype=np.float32), (P, P)).copy()
    shared = {
        "w1n": np.asarray(inputs["W1_neigh"], np.float32),
        "w1s": np.asarray(inputs["W1_self"], np.float32),
        "w2n": np.asarray(inputs["W2_neigh"], np.float32),
        "w2s": np.asarray(inputs["W2_self"], np.float32),
        "iota": iota,
        "b1r": np.broadcast_to(b1, (P, HID_F)).copy(),
        "b2r": np.broadcast_to(b2, (P, OUT_F)).copy(),
    }
    in_maps = []
    for c in range(CORES):
        srcT, dstT, xT, invd = percore[c]
        m = dict(shared)
        m.update({"srcT": srcT, "dstT": dstT, "xT": xT, "invd": invd})
        in_maps.append(m)

    res = run_bass_kernel_spmd(nc, in_maps, list(range(CORES)),
                               trace=trace, tmpdir=tmpdir)
    h2_new = np.concatenate([res.results[c]["out"] for c in range(CORES)], axis=0)
    out = h2_new[pos]
    return out.astype(np.float32), res


def kernel(**inputs) -> np.ndarray:
    out, _ = _run(inputs, trace=False)
    return out
